# revision 2
# baseline (speedup 1.0000x reference)
"""MoE QKV parallel linear for Trainium2, 8 NeuronCores.

Problem: out[t] = x[t] @ W[id[t]].T with x [16384, 2048] f32,
W [4, 3072, 2048] f32, id sorted int32 (tokens pre-grouped by expert).

Sharding: data-parallel over tokens with expert-pure shards. Since tokens
are sorted by expert, split each expert's contiguous token range across a
proportional share of the 8 cores. Every core then runs one dense matmul
[T_max, 2048] @ [2048, 3072] against a single expert's weight (padded with
zero tokens up to the common T_max), which keeps the SPMD program uniform
across cores. Host transposes x-shards and W so the device kernel needs no
on-chip transposes, and scatters the per-core results back.

Device kernel (per core): x^T resident in SBUF (f32r), W^T streamed in
[128, 512] k-tiles, PE accumulates 16 k-tiles per [128 tok, 512 out] PSUM
tile using the fp32r fast path (1 col/cycle), DVE copies PSUM->SBUF,
HWDGE DMAs store to DRAM.
"""

import numpy as np

import concourse.bacc as bacc
import concourse.mybir as mybir
import concourse.tile as tile
from concourse.bass_utils import run_bass_kernel_spmd

NCORES = 8
HIDDEN = 2048
QKV_OUT = 3072
P = 128
KO = HIDDEN // P          # 16 contraction tiles
NCHUNK = 512              # PSUM free dim per matmul (fp32 max)
NCH = QKV_OUT // NCHUNK   # 6 output chunks
MB = 4                    # m-tiles per x DMA block (512 tokens)

_cache: dict = {}


def _build(mt: int):
    """Bass module for one core: out[mt*128, 3072] = xT.T @ wT."""
    nc = bacc.Bacc("TRN2", target_bir_lowering=False, debug=False)
    tmax = mt * P
    f32r = mybir.dt.float32r
    f32 = mybir.dt.float32

    xT = nc.dram_tensor("xT", [HIDDEN, tmax], f32r, kind="ExternalInput")
    wT = nc.dram_tensor("wT", [HIDDEN, QKV_OUT], f32r, kind="ExternalInput")
    out = nc.dram_tensor("out", [tmax, QKV_OUT], f32, kind="ExternalOutput")

    n_mb = mt // MB
    rem = mt % MB

    with tile.TileContext(nc) as tc:
        with (
            tc.tile_pool(name="xa", bufs=max(1, KO * n_mb)) as xa,
            tc.tile_pool(name="xr", bufs=KO if rem else 1) as xr,
            tc.tile_pool(name="wp", bufs=20) as wp,
            tc.tile_pool(name="pp", bufs=6, space="PSUM") as pp,
            tc.tile_pool(name="op", bufs=4) as op,
        ):
            # resident x^T: tiles[ko][mb] of [128, MB*128] (+ ragged tail)
            xt = []
            for ko in range(KO):
                row = []
                for mb in range(n_mb):
                    t = xa.tile([P, MB * P], f32r, name=f"x_{ko}_{mb}", tag="x")
                    nc.sync.dma_start(
                        out=t[:],
                        in_=xT[ko * P:(ko + 1) * P, mb * MB * P:(mb + 1) * MB * P],
                    )
                    row.append(t)
                if rem:
                    t = xr.tile([P, rem * P], f32r, name=f"x_{ko}_r", tag="xr")
                    nc.sync.dma_start(
                        out=t[:], in_=xT[ko * P:(ko + 1) * P, n_mb * MB * P:tmax]
                    )
                    row.append(t)
                xt.append(row)

            def x_slice(ko, m):
                mb, mi = divmod(m, MB)
                return xt[ko][mb][:, mi * P:(mi + 1) * P]

            for n in range(NCH):
                wts = []
                for ko in range(KO):
                    w = wp.tile([P, NCHUNK], f32r, name=f"w_{n}_{ko}", tag="w")
                    nc.sync.dma_start(
                        out=w[:],
                        in_=wT[ko * P:(ko + 1) * P, n * NCHUNK:(n + 1) * NCHUNK],
                    )
                    wts.append(w)
                for m in range(mt):
                    ps = pp.tile([P, NCHUNK], f32, name=f"ps_{n}_{m}", tag="ps")
                    for ko in range(KO):
                        nc.tensor.matmul(
                            ps[:], x_slice(ko, m), wts[ko][:],
                            start=(ko == 0), stop=(ko == KO - 1),
                        )
                    ot = op.tile([P, NCHUNK], f32, name=f"o_{n}_{m}", tag="o")
                    nc.vector.tensor_copy(ot[:], ps[:])
                    nc.scalar.dma_start(
                        out=out[m * P:(m + 1) * P, n * NCHUNK:(n + 1) * NCHUNK],
                        in_=ot[:],
                    )
    nc.compile()
    return nc


def _plan(counts):
    """Allocate 8 cores to experts proportionally (largest remainder),
    then split each expert's token range into per-core contiguous spans.
    Returns (spans, t_max): spans[c] = (expert, start, length)."""
    total = int(counts.sum())
    ne = len(counts)
    active = [e for e in range(ne) if counts[e] > 0]
    quota = {e: counts[e] * NCORES / total for e in active}
    alloc = {e: max(1, int(quota[e])) for e in active}
    while sum(alloc.values()) > NCORES:  # too many mins; shrink largest
        shrinkable = [e for e in active if alloc[e] > 1]
        e = max(shrinkable, key=lambda e: alloc[e] - quota[e])
        alloc[e] -= 1
    rema = sorted(active, key=lambda e: quota[e] - alloc[e], reverse=True)
    i = 0
    while sum(alloc.values()) < NCORES:
        alloc[rema[i % len(rema)]] += 1
        i += 1
    spans = []
    starts = np.concatenate([[0], np.cumsum(counts)])
    for e in active:
        k = alloc[e]
        base, extra = divmod(int(counts[e]), k)
        off = int(starts[e])
        for j in range(k):
            ln = base + (1 if j < extra else 0)
            spans.append((e, off, ln))
            off += ln
    t_max = max(ln for _, _, ln in spans)
    t_max = max(P, -(-t_max // P) * P)
    return spans, t_max


def kernel(x, W, modality_mapping):
    x = np.ascontiguousarray(np.asarray(x, dtype=np.float32))
    W = np.asarray(W, dtype=np.float32)
    mm = np.asarray(modality_mapping)
    T = x.shape[0]
    E = W.shape[0]

    counts = np.bincount(mm.astype(np.int64), minlength=E)
    spans, t_max = _plan(counts)
    mt = t_max // P

    if mt not in _cache:
        _cache[mt] = _build(mt)
    nc = _cache[mt]

    wTs = {}
    in_maps = []
    for e, off, ln in spans:
        if e not in wTs:
            wTs[e] = np.ascontiguousarray(W[e].T)
        xTp = np.zeros((HIDDEN, t_max), dtype=np.float32)
        xTp[:, :ln] = x[off:off + ln].T
        in_maps.append({"xT": xTp, "wT": wTs[e]})

    res = run_bass_kernel_spmd(nc, in_maps, core_ids=list(range(NCORES)))

    out = np.empty((T, QKV_OUT), dtype=np.float32)
    for c, (e, off, ln) in enumerate(spans):
        out[off:off + ln] = res.results[c]["out"][:ln]
    return out


# revision 3
# speedup vs baseline: 1.0949x; 1.0949x over previous
"""MoE QKV parallel linear for Trainium2, 8 NeuronCores.

Problem: out[t] = x[t] @ W[id[t]].T with x [16384, 2048] f32,
W [4, 3072, 2048] f32, id sorted int32 (tokens pre-grouped by expert).

Sharding: data-parallel over tokens with expert-pure shards. Since tokens
are sorted by expert, split each expert's contiguous token range across a
proportional share of the 8 cores. Every core then runs one dense matmul
[T_max, 2048] @ [2048, 3072] against a single expert's weight (padded with
zero tokens up to the common T_max), which keeps the SPMD program uniform
across cores. Host transposes x-shards and W so the device kernel needs no
on-chip transposes, and scatters the per-core results back.

Device kernel (per core): x^T resident in SBUF (f32r), W^T streamed in
[128, 512] k-tiles, PE accumulates 16 k-tiles per [128 tok, 512 out] PSUM
tile using the fp32r fast path (1 col/cycle), DVE copies PSUM->SBUF,
HWDGE DMAs store to DRAM.
"""

import numpy as np

import concourse.bacc as bacc
import concourse.mybir as mybir
import concourse.tile as tile
from concourse.bass_utils import run_bass_kernel_spmd

NCORES = 8
HIDDEN = 2048
QKV_OUT = 3072
P = 128
KO = HIDDEN // P          # 16 contraction tiles
NCHUNK = 512              # PSUM free dim per matmul (fp32 max)
NCH = QKV_OUT // NCHUNK   # 6 output chunks
MB = 4                    # m-tiles per x DMA block (512 tokens)

_cache: dict = {}


def _build(mt: int):
    """Bass module for one core: out[mt*128, 3072] = xT.T @ wT."""
    nc = bacc.Bacc("TRN2", target_bir_lowering=False, debug=False)
    tmax = mt * P
    f32r = mybir.dt.float32r
    f32 = mybir.dt.float32

    xT = nc.dram_tensor("xT", [HIDDEN, tmax], f32r, kind="ExternalInput")
    wT = nc.dram_tensor("wT", [HIDDEN, QKV_OUT], f32r, kind="ExternalInput")
    out = nc.dram_tensor("out", [tmax, QKV_OUT], f32, kind="ExternalOutput")

    n_mb = mt // MB
    rem = mt % MB

    with tile.TileContext(nc) as tc:
        with (
            tc.tile_pool(name="xa", bufs=max(1, KO * n_mb)) as xa,
            tc.tile_pool(name="xr", bufs=KO if rem else 1) as xr,
            tc.tile_pool(name="wp", bufs=22) as wp,
            tc.tile_pool(name="pp", bufs=6, space="PSUM") as pp,
            tc.tile_pool(name="op", bufs=4) as op,
        ):
            # resident x^T: tiles[ko][mb] of [128, MB*128] (+ ragged tail).
            # mb-major emission so the sync ring delivers whole m-blocks in
            # order and matmuls can start after the first block lands.
            xt = [[] for _ in range(KO)]
            for mb in range(n_mb):
                for ko in range(KO):
                    t = xa.tile([P, MB * P], f32r, name=f"x_{ko}_{mb}", tag="x")
                    nc.sync.dma_start(
                        out=t[:],
                        in_=xT[ko * P:(ko + 1) * P, mb * MB * P:(mb + 1) * MB * P],
                    )
                    xt[ko].append(t)
            if rem:
                for ko in range(KO):
                    t = xr.tile([P, rem * P], f32r, name=f"x_{ko}_r", tag="xr")
                    nc.sync.dma_start(
                        out=t[:], in_=xT[ko * P:(ko + 1) * P, n_mb * MB * P:tmax]
                    )
                    xt[ko].append(t)

            def x_slice(ko, m):
                mb, mi = divmod(m, MB)
                return xt[ko][mb][:, mi * P:(mi + 1) * P]

            # W k-tiles ride the scalar (ACT) HWDGE ring so they never queue
            # behind the x stream; chunk c+2's tiles are emitted after chunk
            # c's stores for a distance-2 ring prefetch.
            def load_w(n):
                wts = []
                for ko in range(KO):
                    w = wp.tile([P, NCHUNK], f32r, name=f"w_{n}_{ko}", tag="w")
                    nc.scalar.dma_start(
                        out=w[:],
                        in_=wT[ko * P:(ko + 1) * P, n * NCHUNK:(n + 1) * NCHUNK],
                    )
                    wts.append(w)
                return wts

            wq = {0: load_w(0), 1: load_w(1)}
            for n in range(NCH):
                wts = wq.pop(n)
                for m in range(mt):
                    ps = pp.tile([P, NCHUNK], f32, name=f"ps_{n}_{m}", tag="ps")
                    for ko in range(KO):
                        nc.tensor.matmul(
                            ps[:], x_slice(ko, m), wts[ko][:],
                            start=(ko == 0), stop=(ko == KO - 1),
                        )
                    ot = op.tile([P, NCHUNK], f32, name=f"o_{n}_{m}", tag="o")
                    nc.vector.tensor_copy(ot[:], ps[:])
                    nc.scalar.dma_start(
                        out=out[m * P:(m + 1) * P, n * NCHUNK:(n + 1) * NCHUNK],
                        in_=ot[:],
                    )
                if n + 2 < NCH:
                    wq[n + 2] = load_w(n + 2)
    nc.compile()
    return nc


def _plan(counts):
    """Allocate 8 cores to experts proportionally (largest remainder),
    then split each expert's token range into per-core contiguous spans.
    Returns (spans, t_max): spans[c] = (expert, start, length)."""
    total = int(counts.sum())
    ne = len(counts)
    active = [e for e in range(ne) if counts[e] > 0]
    quota = {e: counts[e] * NCORES / total for e in active}
    alloc = {e: max(1, int(quota[e])) for e in active}
    while sum(alloc.values()) > NCORES:  # too many mins; shrink largest
        shrinkable = [e for e in active if alloc[e] > 1]
        e = max(shrinkable, key=lambda e: alloc[e] - quota[e])
        alloc[e] -= 1
    rema = sorted(active, key=lambda e: quota[e] - alloc[e], reverse=True)
    i = 0
    while sum(alloc.values()) < NCORES:
        alloc[rema[i % len(rema)]] += 1
        i += 1
    spans = []
    starts = np.concatenate([[0], np.cumsum(counts)])
    for e in active:
        k = alloc[e]
        base, extra = divmod(int(counts[e]), k)
        off = int(starts[e])
        for j in range(k):
            ln = base + (1 if j < extra else 0)
            spans.append((e, off, ln))
            off += ln
    t_max = max(ln for _, _, ln in spans)
    t_max = max(P, -(-t_max // P) * P)
    return spans, t_max


def kernel(x, W, modality_mapping):
    x = np.ascontiguousarray(np.asarray(x, dtype=np.float32))
    W = np.asarray(W, dtype=np.float32)
    mm = np.asarray(modality_mapping)
    T = x.shape[0]
    E = W.shape[0]

    counts = np.bincount(mm.astype(np.int64), minlength=E)
    spans, t_max = _plan(counts)
    mt = t_max // P

    if mt not in _cache:
        _cache[mt] = _build(mt)
    nc = _cache[mt]

    wTs = {}
    in_maps = []
    for e, off, ln in spans:
        if e not in wTs:
            wTs[e] = np.ascontiguousarray(W[e].T)
        xTp = np.zeros((HIDDEN, t_max), dtype=np.float32)
        xTp[:, :ln] = x[off:off + ln].T
        in_maps.append({"xT": xTp, "wT": wTs[e]})

    res = run_bass_kernel_spmd(nc, in_maps, core_ids=list(range(NCORES)))

    out = np.empty((T, QKV_OUT), dtype=np.float32)
    for c, (e, off, ln) in enumerate(spans):
        out[off:off + ln] = res.results[c]["out"][:ln]
    return out
